# revision 20
# baseline (speedup 1.0000x reference)
"""Trainium2 Bass kernel for nn_CluTSPSolver (cluster-pointer attention step).

Self-contained: accepts FULL inputs (B=256), shards batch across 8 NeuronCores
(32 per core), runs one SPMD Bass/Tile kernel, gathers full outputs.

Returns (init_aug[B,1,4D] f32, init_guidance_emb[B,1,D] f32,
         init_guidance[B] int32, clu_prob[B,C+1] f32).
"""

from contextlib import ExitStack

import numpy as np

import concourse.bass as bass
import concourse.mybir as mybir
import concourse.tile as tile
from concourse import bacc
from concourse.bass_utils import run_bass_kernel_spmd

# ---- problem dims (hardcoded per spec) ----
B = 256
NCORES = 8
BL = B // NCORES          # 32 local batch
N = 1024                  # nodes
C1 = 101                  # clusters + depot
D = 256
H = 16
QD = D // H               # 16
CLIP = 10.0
NEG = -1000000000.0       # -1e9, exactly representable in f32
NEGS = -4.0e9             # scores-path mask (x0.25 at exp -> -1e9 exact)

F32 = mybir.dt.float32
F32R = mybir.dt.float32r
U8 = mybir.dt.uint8
U32 = mybir.dt.uint32
I32 = mybir.dt.int32

# fp32r (fast PE path) only on the mean-reduction matmuls (moving dim 256)
MEAN_FAST = True


def _r(ap):
    return ap.bitcast(F32R) if MEAN_FAST else ap


def build_kernel():
    """Build + compile the 8-core SPMD bass program. Returns nc."""
    nc = bacc.Bacc(
        "TRN2",
        target_bir_lowering=False,
        debug=False,
        num_devices=NCORES,
    )

    t = {}
    for name, shape, dt in [
        ("dep", [BL, D], F32), ("cur", [BL, D], F32),
        ("clu", [BL, C1, D], F32), ("emb", [BL, N, D], F32R),
        ("mask", [BL, N], U8), ("cmask", [BL, N], U8), ("vcm", [BL, C1], U8),
        ("Wq", [3 * D, D], F32), ("Wk", [D, D], F32), ("Wv", [D, D], F32),
        ("Wks", [D, D], F32), ("Wo", [D, D], F32),
        ("ident", [128, 128], F32), ("M0", [128, 2 * H], F32),
        ("ones16", [1, H], F32),
        ("iota101", [BL, 1], I32), ("coll32", [1, BL * BL], F32),
    ]:
        t[name] = nc.dram_tensor(name, shape, dt, kind="ExternalInput").ap()
    for name, shape, dt in [
        ("init_aug", [BL, 4 * D], F32), ("init_gemb", [BL, D], F32),
        ("init_guid", [BL, 1], I32), ("clu_prob", [BL, C1], F32),
    ]:
        t[name] = nc.dram_tensor(name, shape, dt, kind="ExternalOutput").ap()
    # internal bounce buffer for partition-collapse of the scores mask bias
    t["mb_bounce"] = nc.dram_tensor("mb_bounce", [BL, C1], F32).ap()

    with tile.TileContext(nc) as tc, ExitStack() as ctx:
        _body(ctx, tc, t)
    nc.compile()
    return nc


def _body(ctx, tc, t):
    nc = tc.nc
    sync = nc.sync          # HWDGE dma
    vec = nc.vector
    act = nc.scalar
    pe = nc.tensor
    gps = nc.gpsimd
    Alu = mybir.AluOpType
    Act = mybir.ActivationFunctionType
    Ax = mybir.AxisListType

    # persistent sbuf tensors
    P = ctx.enter_context(tc.tile_pool(name="persist", bufs=1))

    # ------------- phase 0: constants / weights / small inputs -------------
    ident = P.tile([128, 128], F32, tag="ident")
    sync.dma_start(ident[:], t["ident"][:])
    m0 = P.tile([128, 2 * H], F32, tag="m0")
    sync.dma_start(m0[:], t["M0"][:])
    ones16 = P.tile([1, H], F32, tag="ones16")
    sync.dma_start(ones16[:], t["ones16"][:])
    iota101 = P.tile([BL, 1], I32, tag="iota101")
    sync.dma_start(iota101[:], t["iota101"][:])
    coll32 = P.tile([1, BL * BL], F32, tag="coll32")
    sync.dma_start(coll32[:], t["coll32"][:])

    cur_sb = P.tile([BL, D], F32, tag="cur")
    sync.dma_start(cur_sb[:], t["cur"][:])
    dep_sb = P.tile([BL, D], F32, tag="dep")
    sync.dma_start(dep_sb[:], t["dep"][:])

    wq_sb = P.tile([128, 6, D], F32, tag="wq")       # Wq row ic*128+p -> [p, ic, :]
    sync.dma_start(wq_sb[:], t["Wq"].rearrange("(i p) d -> p i d", p=128))
    wk_sb = P.tile([128, 2, D], F32, tag="wk")
    sync.dma_start(wk_sb[:], t["Wk"].rearrange("(i p) d -> p i d", p=128))
    wv_sb = P.tile([128, 2, D], F32, tag="wv")
    sync.dma_start(wv_sb[:], t["Wv"].rearrange("(i p) d -> p i d", p=128))
    wks_sb = P.tile([128, 2, D], F32, tag="wks")
    sync.dma_start(wks_sb[:], t["Wks"].rearrange("(i p) d -> p i d", p=128))
    wo_sb = P.tile([128, 2, D], F32, tag="wo")
    sync.dma_start(wo_sb[:], t["Wo"].rearrange("(i p) d -> p i d", p=128))

    mask_sb = P.tile([BL, N], U8, tag="masku8")
    sync.dma_start(mask_sb[:], t["mask"][:])
    cmask_sb = P.tile([BL, N], U8, tag="cmasku8")
    sync.dma_start(cmask_sb[:], t["cmask"][:])
    vcm_sb = P.tile([BL, C1], U8, tag="vcmu8")
    sync.dma_start(vcm_sb[:], t["vcm"][:])

    # full cluster embedding, c on partitions: [c, b, d]
    clu_sb = P.tile([C1, BL, D], F32, tag="clu")
    sync.dma_start(clu_sb[:], t["clu"].rearrange("b c d -> c b d"))

    # ------------- mask-bias tensors -------------
    # mbS[b,c] = vm[b,c] * NEGS with depot col0 = (~all_real)*NEGS
    mbS = P.tile([BL, C1], F32, tag="mbS")
    mbL = P.tile([BL, C1], F32, tag="mbL")
    vec.tensor_scalar(mbS[:, :], vcm_sb[:, :], NEGS, None, Alu.mult)
    maxr = P.tile([BL, 1], F32, tag="maxr")
    vec.tensor_reduce(maxr[:], mbS[:, 1:C1], Ax.X, Alu.max)
    # col0 = NEGS - maxr   (maxr==0 -> all clusters visited? no: maxr==NEGS means
    #   some cluster NOT visited... mbS=vcm*NEGS: visited->NEGS. max over cols is 0
    #   if any unvisited, NEGS if all visited. col0 masked iff not all visited:
    #   col0 = NEGS - maxr: maxr=0 -> NEGS (masked), maxr=NEGS -> 0 (allowed).
    vec.tensor_scalar(mbS[:, 0:1], maxr[:], -1.0, NEGS, Alu.mult, Alu.add)
    vec.tensor_scalar(mbL[:, :], mbS[:, :], 0.25, None, Alu.mult)
    # flatten mbS to partition 0 via DRAM bounce: mb_p0[0, b*101+c]
    sync.dma_start(t["mb_bounce"][:], mbS[:, :])
    mb_p0 = P.tile([1, BL * C1], F32, tag="mb_p0")
    sync.dma_start(mb_p0[:], t["mb_bounce"].rearrange("b c -> (b c)")[None, :])

    # ------------- keeps (pre-scaled by 1/denom) + keepT -------------
    keep1 = P.tile([BL, N], F32, tag="keep1")   # (~mask) / den1
    keep2 = P.tile([BL, N], F32, tag="keep2")   # (~(mask|cmask)) / den2
    orm = P.tile([BL, N], U8, tag="orm")
    vec.tensor_scalar(keep1[:, :], mask_sb[:, :], -1.0, 1.0, Alu.mult, Alu.add)
    vec.tensor_tensor(orm[:, :], mask_sb[:, :], cmask_sb[:, :], Alu.max)
    vec.tensor_scalar(keep2[:, :], orm[:, :], -1.0, 1.0, Alu.mult, Alu.add)
    for ki, ksrc in ((1, keep1), (2, keep2)):
        dn = P.tile([BL, 1], F32, name=f"dn{ki}", tag=f"dn{ki}")
        vec.tensor_reduce(dn[:], ksrc[:, :], Ax.X, Alu.add)
        vec.tensor_scalar(dn[:], dn[:], 1.0, None, Alu.max)
        vec.reciprocal(dn[:], dn[:])
        vec.tensor_scalar(ksrc[:, :], ksrc[:, :], dn[:], None, Alu.mult)

    keepT = P.tile([128, 8, 2 * BL], F32R, tag="keepT")  # col j*32+b
    uvT = P.tile([128, 2, 2 * BL], F32, tag="uvT")      # [d%128, dc, j*32+b]
    uvc_rows = P.tile([BL, D], F32, tag="uvc_rows")

    with tc.tile_pool(name="ps_t", bufs=2, space="PSUM") as ps_t, \
         tc.tile_pool(name="ps_mean", bufs=2, space="PSUM") as ps_mean, \
         tc.tile_pool(name="upsb", bufs=2) as upsb_pool, \
         tc.tile_pool(name="embp", bufs=3) as emb_pool:

        for j, ksrc in ((0, keep1), (1, keep2)):
            for c in range(8):
                tp = ps_t.tile([128, BL], F32, tag="tp")
                pe.transpose(tp[0:128, 0:BL], ksrc[:, c * 128:(c + 1) * 128],
                             ident[0:BL, 0:BL])
                vec.tensor_copy(keepT[:, c, j * BL:(j + 1) * BL], tp[0:128, 0:BL])

        # masked means over node embeddings (the big DMA stream).
        # up[64, 256] rows {b, 32+b} are the (already normalized) uv/uvc for b.
        for b in range(BL):
            et = emb_pool.tile([128, 8, D], F32R, tag="emb")
            sync.dma_start(et[:], t["emb"][b].rearrange("(f p) d -> p f d", p=128))
            up = ps_mean.tile([2 * BL, D], F32, tag="uvps")
            for c in range(8):
                pe.matmul(up[:], keepT[:, c, :], et[:, c, :],
                          start=(c == 0), stop=(c == 7))
            upc = upsb_pool.tile([2 * BL, D], F32, tag="upc")
            vec.tensor_copy(upc[:], up[:])
            # transpose both d-chunks; keep only columns {b, 32+b}
            for dc in range(2):
                tp = ps_t.tile([128, 2 * BL], F32, tag="tp")
                pe.transpose(tp[0:128, 0:2 * BL], upc[:, dc * 128:(dc + 1) * 128],
                             ident[0:2 * BL, 0:2 * BL])
                vec.tensor_copy(uvT[:, dc, b:2 * BL:BL], tp[0:128, b:2 * BL:BL])

        # uvc rows (partitions 0..31) for init_aug, via transpose-back
        for dc in range(2):
            tp = ps_t.tile([BL, 128], F32, tag="tpc", name=f"tpc_{dc}")
            pe.transpose(tp[0:BL, 0:128], uvT[:, dc, BL:2 * BL], ident[:])
            vec.tensor_copy(uvc_rows[:, dc * 128:(dc + 1) * 128], tp[0:BL, 0:128])

        # WkT / WksT (for qk and w2), curT/depT
        wkT2 = P.tile([128, 2, D], F32, tag="wkT2")    # [d%128, dc, j]
        wksT2 = P.tile([128, 2, D], F32, tag="wksT2")
        for (src, dst) in ((wk_sb, wkT2), (wks_sb, wksT2)):
            for jc in range(2):
                for dc in range(2):
                    tp = ps_t.tile([128, 128], F32, tag="tp2")
                    pe.transpose(tp[:], src[:, jc, dc * 128:(dc + 1) * 128], ident[:])
                    vec.tensor_copy(dst[:, dc, jc * 128:(jc + 1) * 128], tp[:])
        curT = P.tile([128, 2, BL], F32, tag="curT")
        depT = P.tile([128, 2, BL], F32, tag="depT")
        for (src, dst) in ((cur_sb, curT), (dep_sb, depT)):
            for dc in range(2):
                tp = ps_t.tile([128, BL], F32, tag="tp")
                pe.transpose(tp[0:128, 0:BL], src[:, dc * 128:(dc + 1) * 128],
                             ident[0:BL, 0:BL])
                vec.tensor_copy(dst[:, dc, :], tp[0:128, 0:BL])

    # ------------- q (batched) + Qbd + qk -------------
    qT = P.tile([128, 2, BL], F32, tag="qT")
    qbd = P.tile([128, 2, BL * H], F32, tag="qbd")
    qk = P.tile([128, 2, BL * H], F32, tag="qk")
    ctx_chunks = [uvT[:, 0, 0:BL], uvT[:, 1, 0:BL],
                  curT[:, 0, :], curT[:, 1, :],
                  depT[:, 0, :], depT[:, 1, :]]
    with tc.tile_pool(name="ps_q", bufs=2, space="PSUM") as ps_q, \
         tc.tile_pool(name="ps_qk", bufs=2, space="PSUM") as ps_qk:
        for dc in range(2):
            qp = ps_q.tile([128, BL], F32, tag="qps")
            for ic in range(6):
                pe.matmul(qp[:], wq_sb[:, ic, dc * 128:(dc + 1) * 128], ctx_chunks[ic],
                          start=(ic == 0), stop=(ic == 5))
            vec.tensor_copy(qT[:, dc, :], qp[:])
        for dc in range(2):
            for b in range(BL):
                vec.tensor_scalar(qbd[:, dc, b * H:(b + 1) * H],
                                  m0[:, dc * H:(dc + 1) * H],
                                  qT[:, dc, b:b + 1], None, Alu.mult)
        # qk[j,h] per b; single accumulation group per psum bank, LDW amortized
        for jc in range(2):
            qkp = ps_qk.tile([128, BL * H], F32, tag="qkps")
            for dc in range(2):
                for b in range(BL):
                    pe.matmul(qkp[:, b * H:(b + 1) * H],
                              wkT2[:, dc, jc * 128:(jc + 1) * 128],
                              qbd[:, dc, b * H:(b + 1) * H],
                              start=(dc == 0 and b == 0),
                              stop=(dc == 1 and b == BL - 1),
                              skip_group_check=True)
            vec.tensor_copy(qk[:, jc, :], qkp[:])

    # ------------- clusterT + scores/softmax/attnT -------------
    cluT = P.tile([128, 2, BL * C1], F32, tag="cluT")  # [j%128, jc, b*101+c]
    attnT = P.tile([C1, BL * H], F32, tag="attnT")
    with tc.tile_pool(name="ps_ct", bufs=2, space="PSUM") as ps_ct, \
         tc.tile_pool(name="ps_sc", bufs=2, space="PSUM") as ps_sc, \
         tc.tile_pool(name="ps_at", bufs=2, space="PSUM") as ps_at, \
         tc.tile_pool(name="attnp", bufs=3) as attn_pool:
        for b in range(BL):
            for jc in range(2):
                tp = ps_ct.tile([128, C1], F32, tag="ct")
                pe.transpose(tp[0:128, 0:C1], clu_sb[:, b, jc * 128:(jc + 1) * 128],
                             ident[0:C1, 0:C1])
                vec.tensor_copy(cluT[:, jc, b * C1:(b + 1) * C1], tp[0:128, 0:C1])
        for b in range(BL):
            sc = ps_sc.tile([H, C1], F32, tag="scps")
            for jc in range(2):
                pe.matmul(sc[:], qk[:, jc, b * H:(b + 1) * H],
                          cluT[:, jc, b * C1:(b + 1) * C1],
                          start=(jc == 0), stop=False)
            pe.matmul(sc[:], ones16[:], mb_p0[:, b * C1:(b + 1) * C1],
                      start=False, stop=True)
            mx = attn_pool.tile([H, 1], F32, tag="mx")
            vec.tensor_reduce(mx[:], sc[:], Ax.X, Alu.max)
            nmx = attn_pool.tile([H, 1], F32, tag="nmx")
            vec.tensor_scalar(nmx[:], mx[:], -0.25, None, Alu.mult)
            an = attn_pool.tile([H, C1], F32, tag="an")
            se = attn_pool.tile([H, 1], F32, tag="se")
            act.activation(an[:], sc[:], Act.Exp, bias=nmx[:], scale=0.25,
                           accum_out=se[:])
            rs = attn_pool.tile([H, 1], F32, tag="rs")
            vec.reciprocal(rs[:], se[:])
            vec.tensor_scalar(an[:], an[:], rs[:], None, Alu.mult)
            tp = ps_at.tile([C1, H], F32, tag="at")
            pe.transpose(tp[0:C1, 0:H], an[:], ident[0:H, 0:H])
            vec.tensor_copy(attnT[:, b * H:(b + 1) * H], tp[0:C1, 0:H])

    # ------------- zT, glimpseT (diag extract), outG_T, w2 -------------
    zT = P.tile([128, 2, BL * H], F32, tag="zT")
    glT = P.tile([128, 2, BL], F32, tag="glT")
    ogT = P.tile([128, 2, BL], F32, tag="ogT")
    w2 = P.tile([128, 2, BL], F32, tag="w2")
    with tc.tile_pool(name="ps_z", bufs=3, space="PSUM") as ps_z, \
         tc.tile_pool(name="ps_g", bufs=2, space="PSUM") as ps_g, \
         tc.tile_pool(name="scrp", bufs=2) as scratch_pool:
        for b in range(BL):
            for jc in range(2):
                zp = ps_z.tile([128, H], F32, tag="zps")
                pe.matmul(zp[:], clu_sb[:, b, jc * 128:(jc + 1) * 128],
                          attnT[:, b * H:(b + 1) * H], start=True, stop=True)
                vec.tensor_copy(zT[:, jc, b * H:(b + 1) * H], zp[:])
        for dc in range(2):
            gp = ps_g.tile([128, BL * H], F32, tag="gps")
            for jc in range(2):
                for b in range(BL):
                    pe.matmul(gp[:, b * H:(b + 1) * H],
                              wv_sb[:, jc, dc * 128:(dc + 1) * 128],
                              zT[:, jc, b * H:(b + 1) * H],
                              start=(jc == 0 and b == 0),
                              stop=(jc == 1 and b == BL - 1),
                              skip_group_check=True)
            for b in range(BL):
                scr = scratch_pool.tile([128, H], F32, tag="scr")
                vec.tensor_tensor(scr[:], gp[:, b * H:(b + 1) * H],
                                  m0[:, dc * H:(dc + 1) * H], Alu.mult)
                vec.tensor_reduce(glT[:, dc, b:b + 1], scr[:], Ax.X, Alu.add)

    with tc.tile_pool(name="ps_o", bufs=2, space="PSUM") as ps_o:
        for dc in range(2):
            op = ps_o.tile([128, BL], F32, tag="ops")
            for ic in range(2):
                pe.matmul(op[:], wo_sb[:, ic, dc * 128:(dc + 1) * 128], glT[:, ic, :],
                          start=(ic == 0), stop=(ic == 1))
            vec.tensor_copy(ogT[:, dc, :], op[:])
        for jc in range(2):
            wp = ps_o.tile([128, BL], F32, tag="w2ps")
            for dc in range(2):
                pe.matmul(wp[:], wksT2[:, dc, jc * 128:(jc + 1) * 128], ogT[:, dc, :],
                          start=(dc == 0), stop=(dc == 1))
            vec.tensor_copy(w2[:, jc, :], wp[:])

    # ------------- logits: per-b M=1 rows, tanh, indicator-collect -------------
    logits = P.tile([BL, C1], F32, tag="logits")
    with tc.tile_pool(name="ps_l", bufs=3, space="PSUM") as ps_l, \
         tc.tile_pool(name="ps_lc", bufs=1, space="PSUM") as ps_lc, \
         tc.tile_pool(name="rowp", bufs=3) as row_pool:
        lcol = ps_lc.tile([BL, C1], F32, tag="lcol")
        for b in range(BL):
            lp = ps_l.tile([1, C1], F32, tag="lp")
            for jc in range(2):
                pe.matmul(lp[:], w2[:, jc, b:b + 1],
                          cluT[:, jc, b * C1:(b + 1) * C1],
                          start=(jc == 0), stop=(jc == 1))
            trow = row_pool.tile([1, C1], F32, tag="trow")
            act.activation(trow[:], lp[:], Act.Tanh, scale=1.0 / 16.0)
            pe.matmul(lcol[:], coll32[:, b * BL:(b + 1) * BL], trow[:],
                      start=(b == 0), stop=(b == BL - 1),
                      skip_group_check=True)
        # logits = 10*tanh + mask-bias, batched
        vec.scalar_tensor_tensor(out=logits[:], in0=lcol[:], scalar=CLIP,
                                 in1=mbL[:, :], op0=Alu.mult, op1=Alu.add)

    mx = P.tile([BL, 1], F32, tag="lmx")
    vec.tensor_reduce(mx[:], logits[:], Ax.X, Alu.max)
    nmx = P.tile([BL, 1], F32, tag="lnmx")
    vec.tensor_scalar(nmx[:], mx[:], -1.0, None, Alu.mult)
    ex = P.tile([BL, C1], F32, tag="lex")
    se = P.tile([BL, 1], F32, tag="lse")
    act.activation(ex[:], logits[:], Act.Exp, bias=nmx[:], scale=1.0, accum_out=se[:])
    ln = P.tile([BL, 1], F32, tag="lln")
    act.activation(ln[:], se[:], Act.Ln)
    nln = P.tile([BL, 1], F32, tag="lnln")
    vec.tensor_scalar(nln[:], ln[:], -1.0, None, Alu.mult)
    logp = P.tile([BL, C1], F32, tag="logp")
    vec.tensor_scalar(logp[:], logits[:], nmx[:], nln[:], Alu.add, Alu.add)
    sync.dma_start(t["clu_prob"][:], logp[:])

    mx8 = P.tile([BL, 8], F32, tag="mx8")
    idx8 = P.tile([BL, 8], U32, tag="idx8")
    vec.max(mx8[:], logits[:])
    vec.max_index(idx8[:], mx8[:], logits[:])
    sync.dma_start(t["init_guid"][:], idx8[:, 0:1].bitcast(I32))

    # gather selected cluster embedding rows from DRAM
    gidx = P.tile([BL, 1], I32, tag="gidx")
    vec.tensor_tensor(gidx[:], idx8[:, 0:1].bitcast(I32), iota101[:], Alu.add)
    selemb = P.tile([BL, D], F32, tag="selemb")
    gps.indirect_dma_start(
        out=selemb[:], out_offset=None,
        in_=t["clu"][:],
        in_offset=bass.IndirectOffsetOnAxis(ap=gidx[:, 0:1], axis=1),
    )
    sync.dma_start(t["init_gemb"][:], selemb[:])

    # init_aug = [uvc, current, sel_emb, depot]
    sync.dma_start(t["init_aug"][:, 0:D], uvc_rows[:])
    sync.dma_start(t["init_aug"][:, D:2 * D], cur_sb[:])
    sync.dma_start(t["init_aug"][:, 2 * D:3 * D], selemb[:])
    sync.dma_start(t["init_aug"][:, 3 * D:4 * D], dep_sb[:])


# ---------------- host side ----------------
def _host_consts():
    ident = np.eye(128, dtype=np.float32)
    m0 = np.zeros((128, 2 * H), dtype=np.float32)
    for p in range(128):
        m0[p, p // QD] = 1.0            # chunk0: h = p//16 (0..7)
        m0[p, H + 8 + p // QD] = 1.0    # chunk1: h = 8 + p//16 (8..15)
    ones16 = np.ones((1, H), dtype=np.float32)
    iota101 = (np.arange(BL, dtype=np.int32) * C1).reshape(BL, 1)
    coll32 = np.eye(BL, dtype=np.float32).reshape(1, BL * BL)
    return ident, m0, ones16, iota101, coll32


def _make_in_maps(depot_embedding, cluster_embedding, current_embedding,
                  node_embeddings, mask, cluster_mask, visited_cluster_mask,
                  Wq, Wk, Wv, Wks, Wo):
    ident, m0, ones16, iota101, coll32 = _host_consts()
    f32 = np.float32
    dep = np.ascontiguousarray(np.asarray(depot_embedding, dtype=f32).reshape(B, D))
    cur = np.ascontiguousarray(np.asarray(current_embedding, dtype=f32).reshape(B, D))
    clu = np.ascontiguousarray(np.asarray(cluster_embedding, dtype=f32))
    emb = np.ascontiguousarray(np.asarray(node_embeddings, dtype=f32))
    msk = np.ascontiguousarray(np.asarray(mask).reshape(B, N).astype(np.uint8))
    cmsk = np.ascontiguousarray(np.asarray(cluster_mask).reshape(B, N).astype(np.uint8))
    vcm = np.ascontiguousarray(
        np.asarray(visited_cluster_mask).reshape(B, C1).astype(np.uint8))
    wq = np.ascontiguousarray(np.asarray(Wq, dtype=f32))
    wk = np.ascontiguousarray(np.asarray(Wk, dtype=f32))
    wv = np.ascontiguousarray(np.asarray(Wv, dtype=f32))
    wks = np.ascontiguousarray(np.asarray(Wks, dtype=f32))
    wo = np.ascontiguousarray(np.asarray(Wo, dtype=f32))

    in_maps = []
    for c in range(NCORES):
        s = slice(c * BL, (c + 1) * BL)
        in_maps.append({
            "dep": dep[s], "cur": cur[s], "clu": clu[s], "emb": emb[s],
            "mask": msk[s], "cmask": cmsk[s], "vcm": vcm[s],
            "Wq": wq, "Wk": wk, "Wv": wv, "Wks": wks, "Wo": wo,
            "ident": ident, "M0": m0, "ones16": ones16,
            "iota101": iota101, "coll32": coll32,
        })
    return in_maps


_NC_CACHE = None


def _get_nc():
    global _NC_CACHE
    if _NC_CACHE is None:
        _NC_CACHE = build_kernel()
    return _NC_CACHE


def _assemble(res):
    init_aug = np.concatenate([r["init_aug"] for r in res], axis=0).reshape(B, 1, 4 * D)
    gemb = np.concatenate([r["init_gemb"] for r in res], axis=0).reshape(B, 1, D)
    guid = np.concatenate([r["init_guid"] for r in res], axis=0).reshape(B).astype(np.int32)
    prob = np.concatenate([r["clu_prob"] for r in res], axis=0).reshape(B, C1)
    return init_aug, gemb, guid, prob


def kernel(**inputs):
    nc = _get_nc()
    in_maps = _make_in_maps(**inputs)
    res = run_bass_kernel_spmd(nc, in_maps, list(range(NCORES))).results
    return _assemble(res)


if __name__ == "__main__":
    build_kernel()
    print("built ok")
